# revision 12
# baseline (speedup 1.0000x reference)
"""Trainium2 Bass kernel for the Batchelor motion-compensated MRI forward model.

out[., x, y, c] = sum_t mask[x,y,c,t] * fft2c( warp(img, flow_t) * smaps[:,:,c] )

Strategy: shard the Nt=24 frames across 8 NeuronCores (3 frames each).
Each core: bilinear warp via per-pixel indirect-DMA quad gather, coil
multiply, centered 2D DFT as fp32r matmuls (fftshifts folded into the DFT
matrix), k-space mask multiply-accumulate.  Host sums the 8 partials.
"""

import numpy as np

Nx = Ny = 256
Nc = 16
Nt = 24
NCORES = 8
TPC = Nt // NCORES  # frames per core

_cache = {}


# ----------------------------------------------------------------- host prep

def _g_matrices():
    # fftshift(fft(ifftshift(x), norm='ortho')) == G @ x with
    # G[k,n] = (-1)^(k+n) * exp(-2i pi k n / N) / sqrt(N)
    k = np.arange(Nx)
    sign = (-1.0) ** (k[:, None] + k[None, :])
    w = np.exp(-2j * np.pi * np.outer(k, k) / Nx) / np.sqrt(Nx)
    G = sign * w
    return G.real.astype(np.float32), G.imag.astype(np.float32)


def _host_constants(image_real, image_imag):
    Gr, Gi = _g_matrices()
    Gn = (-Gi).astype(np.float32)

    # stage A fused moving operands: [variant, x, 512]
    gmatA = np.empty((2, Nx, 2 * Ny), dtype=np.float32)
    gmatA[0, :, :Ny] = Gr
    gmatA[0, :, Ny:] = Gi
    gmatA[1, :, :Ny] = Gn
    gmatA[1, :, Ny:] = Gr
    # stage B stationary planes: [3, y, ky] (Gr, Gi, -Gi)
    gmatB = np.stack([Gr, Gi, Gn], axis=0)

    # quad-interleaved image for the bilinear gather:
    # imgq[x*256+y] = [r(x,y), i(x,y), r(x+1,y), i(x+1,y),
    #                  r(x,y+1), i(x,y+1), r(x+1,y+1), i(x+1,y+1)]
    r = image_real.astype(np.float32)
    im = image_imag.astype(np.float32)
    rx = np.concatenate([r[1:], r[-1:]], axis=0)
    ix = np.concatenate([im[1:], im[-1:]], axis=0)
    ry = np.concatenate([r[:, 1:], r[:, -1:]], axis=1)
    iy = np.concatenate([im[:, 1:], im[:, -1:]], axis=1)
    rxy = np.concatenate([ry[1:], ry[-1:]], axis=0)
    ixy = np.concatenate([iy[1:], iy[-1:]], axis=0)
    imgq = np.stack([r, im, rx, ix, ry, iy, rxy, ixy], axis=-1)
    imgq = np.ascontiguousarray(imgq.reshape(Nx * Ny, 8))

    iotax = np.broadcast_to(
        np.arange(Nx, dtype=np.float32).reshape(2, 128)[:, :, None], (2, 128, Ny)
    )
    iotay = np.broadcast_to(np.arange(Ny, dtype=np.float32)[None, :], (128, Ny))
    return {
        "gmatA": gmatA,
        "gmatB": np.ascontiguousarray(gmatB),
        "imgq": imgq,
        "iotax": np.ascontiguousarray(iotax),
        "iotay": np.ascontiguousarray(iotay),
    }


def _shard_inputs(image_real, image_imag, mask, smaps_real, smaps_imag, flow):
    consts = _host_constants(image_real, image_imag)
    smapsT = np.ascontiguousarray(
        np.stack([smaps_real, smaps_imag], axis=0).transpose(3, 0, 1, 2)
    )  # [c, ri, x, y]
    in_maps = []
    for core in range(NCORES):
        ts = range(core * TPC, (core + 1) * TPC)
        fl = np.ascontiguousarray(
            np.stack([np.stack([flow[:, :, 0, t], flow[:, :, 1, t]]) for t in ts])
        )  # [tt, comp, x, y]
        mk = np.ascontiguousarray(
            np.stack(
                [np.stack([mask[:, :, c, t].T for t in ts]) for c in range(Nc)]
            )
        )  # [c, tt, ky(y), kx(x)]
        m = dict(consts)
        m["flow"] = fl
        m["maskt"] = mk
        m["smapst"] = smapsT
        in_maps.append(m)
    return in_maps


def _unshard(partials):
    # partial: [128, c, ri, m, kx]; ky = m*128 + p
    total = np.zeros_like(partials[0], dtype=np.float64)
    for p in partials:
        total += p
    total = total.astype(np.float32)
    # -> [ri, kx, ky, c]
    t = total.transpose(2, 4, 3, 0, 1)  # [ri, kx, m, p, c]
    t = t.reshape(2, Nx, Ny, Nc)
    return t


# -------------------------------------------------------------- kernel build

def _emit(nc, tc, debug=False):
    import concourse.mybir as mybir
    from concourse.bass import IndirectOffsetOnAxis

    f32 = mybir.dt.float32
    f32r = mybir.dt.float32r
    i32 = mybir.dt.int32
    Alu = mybir.AluOpType

    flow_d = nc.dram_tensor("flow", (TPC, 2, Nx, Ny), f32, kind="ExternalInput").ap()
    maskt_d = nc.dram_tensor("maskt", (Nc, TPC, Ny, Nx), f32, kind="ExternalInput").ap()
    smapst_d = nc.dram_tensor("smapst", (Nc, 2, Nx, Ny), f32, kind="ExternalInput").ap()
    gmatA_d = nc.dram_tensor("gmatA", (2, Nx, 2 * Ny), f32, kind="ExternalInput").ap()
    gmatB_d = nc.dram_tensor("gmatB", (3, Nx, Ny), f32, kind="ExternalInput").ap()
    imgq_d = nc.dram_tensor("imgq", (Nx * Ny, 8), f32, kind="ExternalInput").ap()
    iotax_d = nc.dram_tensor("iotax", (2, 128, Ny), f32, kind="ExternalInput").ap()
    iotay_d = nc.dram_tensor("iotay", (128, Ny), f32, kind="ExternalInput").ap()
    out_d = nc.dram_tensor(
        "out", (128, Nc, 2, 2, Ny), f32, kind="ExternalOutput"
    ).ap()
    if debug:
        dbg_qi = nc.dram_tensor("dbg_qi", (128, 2, Ny), mybir.dt.int32, kind="ExternalOutput").ap()
        dbg_quad = nc.dram_tensor("dbg_quad", (128, 2, Ny, 8), f32, kind="ExternalOutput").ap()
        dbg_W = nc.dram_tensor("dbg_W", (128, TPC, 2, 2, Ny), f32, kind="ExternalOutput").ap()
        dbg_X = nc.dram_tensor("dbg_X", (128, 2, 2, Ny), f32, kind="ExternalOutput").ap()
        dbg_s1 = nc.dram_tensor("dbg_s1", (128, 2, 2 * Ny), f32, kind="ExternalOutput").ap()
        dbg_kt = nc.dram_tensor("dbg_kt", (128, 2, 2 * Ny), f32, kind="ExternalOutput").ap()

    import contextlib

    ctx = contextlib.ExitStack()
    with ctx:
        consts = ctx.enter_context(tc.tile_pool(name="consts", bufs=1))
        accp = ctx.enter_context(tc.tile_pool(name="acc", bufs=1))
        warp = ctx.enter_context(tc.tile_pool(name="warp", bufs=2))
        wres = ctx.enter_context(tc.tile_pool(name="wres", bufs=1))
        unit = ctx.enter_context(tc.tile_pool(name="unit", bufs=2))
        mpool = ctx.enter_context(tc.tile_pool(name="mask", bufs=3))
        psA = ctx.enter_context(tc.tile_pool(name="psA", bufs=2, space="PSUM"))
        psB = ctx.enter_context(tc.tile_pool(name="psB", bufs=2, space="PSUM"))

        # ---- constants into SBUF
        gA = consts.tile([128, 2, 2, 2 * Ny], f32r, tag="gA")  # [p, var, ktile, 512]
        for v in range(2):
            nc.sync.dma_start(
                out=gA[:, v],
                in_=gmatA_d[v].rearrange("(k p) n -> p k n", p=128).bitcast(f32r),
            )
        gB = consts.tile([128, 3, 2, Ny], f32r, tag="gB")  # [p, plane, ktile, ky]
        for pl in range(3):
            nc.sync.dma_start(
                out=gB[:, pl],
                in_=gmatB_d[pl].rearrange("(k p) n -> p k n", p=128).bitcast(f32r),
            )
        iox = consts.tile([128, 2, Ny], f32, tag="iox")
        nc.sync.dma_start(out=iox, in_=iotax_d.rearrange("k p n -> p k n"))
        ioy = consts.tile([128, Ny], f32, tag="ioy")
        nc.sync.dma_start(out=ioy, in_=iotay_d)

        # ---- accumulator
        acc = accp.tile([128, Nc, 2, 2, Ny], f32, tag="acc")

        # ---- phase 1: warp each frame
        W = wres.tile([128, TPC, 2, 2, Ny], f32, tag="W")  # [p, tt, xt, ri, y]
        for tt in range(TPC):
            fx = warp.tile([128, 2, Ny], f32, tag="fx")
            fy = warp.tile([128, 2, Ny], f32, tag="fy")
            nc.sync.dma_start(
                out=fx, in_=flow_d[tt, 0].rearrange("(k p) n -> p k n", p=128)
            )
            nc.sync.dma_start(
                out=fy, in_=flow_d[tt, 1].rearrange("(k p) n -> p k n", p=128)
            )
            quads = []
            wxs = []
            wys = []
            for xt in range(2):
                xc = warp.tile([128, Ny], f32, tag="xc")
                yc = warp.tile([128, Ny], f32, tag="yc")
                nc.vector.tensor_tensor(out=xc, in0=fx[:, xt], in1=iox[:, xt], op=Alu.add)
                nc.vector.tensor_scalar(
                    out=xc, in0=xc, scalar1=0.0, scalar2=float(Nx - 1),
                    op0=Alu.max, op1=Alu.min,
                )
                nc.vector.tensor_tensor(out=yc, in0=fy[:, xt], in1=ioy, op=Alu.add)
                nc.vector.tensor_scalar(
                    out=yc, in0=yc, scalar1=0.0, scalar2=float(Ny - 1),
                    op0=Alu.max, op1=Alu.min,
                )
                # floor(x) = (x + (1.5*2^23 - 0.5)) - 1.5*2^23 under f32 RNE;
                # the tie direction is harmless: off-by-one floor gives w=1,
                # which selects the exact neighbor value in the lerp.
                MAGIC = 12582912.0
                x0 = warp.tile([128, Ny], f32, tag="x0")
                y0 = warp.tile([128, Ny], f32, tag="y0")
                nc.vector.tensor_single_scalar(out=x0, in_=xc, scalar=-0.5, op=Alu.add)
                nc.vector.tensor_single_scalar(out=x0, in_=x0, scalar=MAGIC, op=Alu.add)
                nc.vector.tensor_scalar(
                    out=x0, in0=x0, scalar1=MAGIC, scalar2=float(Nx - 2),
                    op0=Alu.subtract, op1=Alu.min,
                )
                nc.vector.tensor_single_scalar(out=y0, in_=yc, scalar=-0.5, op=Alu.add)
                nc.vector.tensor_single_scalar(out=y0, in_=y0, scalar=MAGIC, op=Alu.add)
                nc.vector.tensor_scalar(
                    out=y0, in0=y0, scalar1=MAGIC, scalar2=float(Ny - 2),
                    op0=Alu.subtract, op1=Alu.min,
                )
                # weights
                wx = warp.tile([128, Ny], f32, tag="wx")
                wy = warp.tile([128, Ny], f32, tag="wy")
                nc.vector.tensor_tensor(out=wx, in0=xc, in1=x0, op=Alu.subtract)
                nc.vector.tensor_tensor(out=wy, in0=yc, in1=y0, op=Alu.subtract)
                wxs.append(wx)
                wys.append(wy)
                # flat quad index
                qf = warp.tile([128, Ny], f32, tag="qf")
                nc.gpsimd.tensor_single_scalar(out=qf, in_=x0, scalar=float(Ny), op=Alu.mult)
                nc.gpsimd.tensor_tensor(out=qf, in0=qf, in1=y0, op=Alu.add)
                qi = warp.tile([128, Ny], i32, tag="qi")
                nc.gpsimd.tensor_copy(out=qi, in_=qf)
                quad = warp.tile([128, Ny, 8], f32, tag="quad")
                # HW honors only one dynamic offset per partition per DMA
                # (scalar_dynamic_offset DGE level): gather column-by-column.
                for j in range(Ny):
                    nc.gpsimd.indirect_dma_start(
                        out=quad[:, j], out_offset=None,
                        in_=imgq_d,
                        in_offset=IndirectOffsetOnAxis(ap=qi[:, j : j + 1], axis=0),
                    )
                if debug and tt == 0:
                    nc.sync.dma_start(out=dbg_qi[:, xt], in_=qi)
                    nc.sync.dma_start(out=dbg_quad[:, xt], in_=quad)
                quads.append(quad)
            for xt in range(2):
                quad, wx, wy = quads[xt], wxs[xt], wys[xt]
                # bilinear lerp -> W
                wu = warp.tile([128, Ny], f32, tag="wu")  # 1-wx
                wv = warp.tile([128, Ny], f32, tag="wv")  # 1-wy
                nc.vector.tensor_scalar(
                    out=wu, in0=wx, scalar1=-1.0, scalar2=1.0, op0=Alu.mult, op1=Alu.add
                )
                nc.vector.tensor_scalar(
                    out=wv, in0=wy, scalar1=-1.0, scalar2=1.0, op0=Alu.mult, op1=Alu.add
                )
                w00 = warp.tile([128, Ny], f32, tag="w00")
                w10 = warp.tile([128, Ny], f32, tag="w10")
                w01 = warp.tile([128, Ny], f32, tag="w01")
                w11 = warp.tile([128, Ny], f32, tag="w11")
                nc.vector.tensor_tensor(out=w00, in0=wu, in1=wv, op=Alu.mult)
                nc.vector.tensor_tensor(out=w10, in0=wx, in1=wv, op=Alu.mult)
                nc.vector.tensor_tensor(out=w01, in0=wu, in1=wy, op=Alu.mult)
                nc.vector.tensor_tensor(out=w11, in0=wx, in1=wy, op=Alu.mult)
                t0 = warp.tile([128, Ny], f32, tag="t0")
                for ri in range(2):
                    wdst = W[:, tt, xt, ri]
                    nc.vector.tensor_tensor(out=wdst, in0=w00, in1=quad[:, :, 0 + ri], op=Alu.mult)
                    nc.vector.tensor_tensor(out=t0, in0=w10, in1=quad[:, :, 2 + ri], op=Alu.mult)
                    nc.vector.tensor_tensor(out=wdst, in0=wdst, in1=t0, op=Alu.add)
                    nc.vector.tensor_tensor(out=t0, in0=w01, in1=quad[:, :, 4 + ri], op=Alu.mult)
                    nc.vector.tensor_tensor(out=wdst, in0=wdst, in1=t0, op=Alu.add)
                    nc.vector.tensor_tensor(out=t0, in0=w11, in1=quad[:, :, 6 + ri], op=Alu.mult)
                    nc.vector.tensor_tensor(out=wdst, in0=wdst, in1=t0, op=Alu.add)

        if debug:
            nc.sync.dma_start(out=dbg_W, in_=W)
        # ---- phase 2: per (coil, frame): coil mult, DFT, mask-MAC
        for c in range(Nc):
            smap = unit.tile([128, 2, 2, Ny], f32, tag="smap")  # [p, ri, xt, y]
            for ri in range(2):
                nc.sync.dma_start(
                    out=smap[:, ri],
                    in_=smapst_d[c, ri].rearrange("(k p) n -> p k n", p=128),
                )
            for tt in range(TPC):
                X = unit.tile([128, 2, 2, Ny], f32r, tag="X")  # [p, xt, ri, y]
                t1 = unit.tile([128, Ny], f32, tag="t1")
                t2 = unit.tile([128, Ny], f32, tag="t2")
                for xt in range(2):
                    wr = W[:, tt, xt, 0]
                    wi = W[:, tt, xt, 1]
                    sr = smap[:, 0, xt]
                    si = smap[:, 1, xt]
                    nc.vector.tensor_tensor(out=t1, in0=wr, in1=sr, op=Alu.mult)
                    nc.vector.tensor_tensor(out=t2, in0=wi, in1=si, op=Alu.mult)
                    nc.vector.tensor_tensor(out=X[:, xt, 0], in0=t1, in1=t2, op=Alu.subtract)
                    nc.vector.tensor_tensor(out=t1, in0=wr, in1=si, op=Alu.mult)
                    nc.vector.tensor_tensor(out=t2, in0=wi, in1=sr, op=Alu.mult)
                    nc.vector.tensor_tensor(out=X[:, xt, 1], in0=t1, in1=t2, op=Alu.add)

                # stage A: S1T[y, kx(r|i)] = sum_x X[x,y] * G[x,kx]
                pa = [psA.tile([128, 2 * Ny], f32, tag=f"psA{m}", name=f"psA{m}") for m in range(2)]
                for m in range(2):
                    ms = slice(m * 128, (m + 1) * 128)
                    for k in range(2):
                        nc.tensor.matmul(
                            pa[m][:],
                            X[:, k, 0, ms],
                            gA[:, 0, k],
                            start=(k == 0), stop=False,
                        )
                        nc.tensor.matmul(
                            pa[m][:],
                            X[:, k, 1, ms],
                            gA[:, 1, k],
                            start=False, stop=(k == 1),
                        )
                s1 = unit.tile([128, 2, 2 * Ny], f32r, tag="s1")  # [p, ytile, kxr|kxi]
                for m in range(2):
                    nc.scalar.copy(out=s1[:, m], in_=pa[m][:])
                if debug and c == 0 and tt == 0:
                    nc.sync.dma_start(out=dbg_X, in_=X.bitcast(f32))
                    nc.sync.dma_start(out=dbg_s1, in_=s1.bitcast(f32))

                # stage B: KT[ky, kx] = sum_y G[y,ky] * S1T[y,kx]
                pb = [psB.tile([128, 2 * Ny], f32, tag=f"psB{m}", name=f"psB{m}") for m in range(2)]
                for m2 in range(2):
                    ms = slice(m2 * 128, (m2 + 1) * 128)
                    # real half: Gr@S1Tr + (-Gi)@S1Ti  (planes 0, 2)
                    # imag half: Gi@S1Tr + Gr@S1Ti     (planes 1, 0)
                    for half, (pl_r, pl_i) in enumerate([(0, 2), (1, 0)]):
                        dst = pb[m2][:, half * Ny : (half + 1) * Ny]
                        for k2 in range(2):
                            nc.tensor.matmul(
                                dst,
                                gB[:, pl_r, k2, ms],
                                s1[:, k2, 0:Ny],
                                start=(k2 == 0), stop=False,
                            )
                            nc.tensor.matmul(
                                dst,
                                gB[:, pl_i, k2, ms],
                                s1[:, k2, Ny : 2 * Ny],
                                start=False, stop=(k2 == 1),
                            )

                if debug and c == 0 and tt == 0:
                    ktd = unit.tile([128, 2, 2 * Ny], f32, tag="ktd")
                    for m2 in range(2):
                        nc.scalar.copy(out=ktd[:, m2], in_=pb[m2][:])
                    nc.sync.dma_start(out=dbg_kt, in_=ktd)
                # mask MAC
                mk = mpool.tile([128, 2, Nx], f32, tag="mk")  # [p, kytile, kx]
                nc.sync.dma_start(
                    out=mk,
                    in_=maskt_d[c, tt].rearrange("(k p) n -> p k n", p=128),
                )
                mt = unit.tile([128, Nx], f32, tag="mt")
                for m2 in range(2):
                    for ri in range(2):
                        src = pb[m2][:, ri * Ny : (ri + 1) * Ny]
                        dst = acc[:, c, ri, m2]
                        if tt == 0:
                            nc.vector.tensor_tensor(out=dst, in0=mk[:, m2], in1=src, op=Alu.mult)
                        else:
                            nc.vector.tensor_tensor(out=mt, in0=mk[:, m2], in1=src, op=Alu.mult)
                            nc.vector.tensor_tensor(out=dst, in0=dst, in1=mt, op=Alu.add)

        # ---- out
        for half in range(2):
            nc.sync.dma_start(
                out=out_d[:, half * 8 : (half + 1) * 8],
                in_=acc[:, half * 8 : (half + 1) * 8],
            )


def _build(debug=False):
    key = ("nc", debug)
    if key in _cache:
        return _cache[key]
    import concourse.bacc as bacc
    import concourse.tile as tile

    nc = bacc.Bacc("TRN2", target_bir_lowering=False, debug=False)
    with tile.TileContext(nc) as tc:
        _emit(nc, tc, debug=debug)
    nc.compile()
    _cache[key] = nc
    return nc


def kernel(
    image_real=None, image_imag=None, mask=None,
    smaps_real=None, smaps_imag=None, flow=None,
):
    from concourse import bass_utils

    image_real = np.asarray(image_real, dtype=np.float32)
    image_imag = np.asarray(image_imag, dtype=np.float32)
    mask = np.asarray(mask, dtype=np.float32)
    smaps_real = np.asarray(smaps_real, dtype=np.float32)
    smaps_imag = np.asarray(smaps_imag, dtype=np.float32)
    flow = np.asarray(flow, dtype=np.float32)

    in_maps = _shard_inputs(image_real, image_imag, mask, smaps_real, smaps_imag, flow)
    nc = _build()
    res = bass_utils.run_bass_kernel_spmd(nc, in_maps, core_ids=list(range(NCORES)))
    partials = [r["out"] for r in res.results]
    return _unshard(partials)


# revision 14
# speedup vs baseline: 1.0727x; 1.0727x over previous
"""Trainium2 Bass kernel for the Batchelor motion-compensated MRI forward model.

out[., x, y, c] = sum_t mask[x,y,c,t] * fft2c( warp(img, flow_t) * smaps[:,:,c] )

Strategy: shard the Nt=24 frames across 8 NeuronCores (3 frames each).
Each core: bilinear warp via per-pixel indirect-DMA quad gather, coil
multiply, centered 2D DFT as fp32r matmuls (fftshifts folded into the DFT
matrix), k-space mask multiply-accumulate.  Host sums the 8 partials.
"""

import numpy as np

Nx = Ny = 256
Nc = 16
Nt = 24
NCORES = 8
TPC = Nt // NCORES  # frames per core

_cache = {}


# ----------------------------------------------------------------- host prep

def _g_matrices():
    # fftshift(fft(ifftshift(x), norm='ortho')) == G @ x with
    # G[k,n] = (-1)^(k+n) * exp(-2i pi k n / N) / sqrt(N)
    k = np.arange(Nx)
    sign = (-1.0) ** (k[:, None] + k[None, :])
    w = np.exp(-2j * np.pi * np.outer(k, k) / Nx) / np.sqrt(Nx)
    G = sign * w
    return G.real.astype(np.float32), G.imag.astype(np.float32)


def _host_constants(image_real, image_imag):
    Gr, Gi = _g_matrices()
    Gn = (-Gi).astype(np.float32)

    # stage A fused moving operands: [variant, x, 512]
    gmatA = np.empty((2, Nx, 2 * Ny), dtype=np.float32)
    gmatA[0, :, :Ny] = Gr
    gmatA[0, :, Ny:] = Gi
    gmatA[1, :, :Ny] = Gn
    gmatA[1, :, Ny:] = Gr
    # stage B stationary planes: [3, y, ky] (Gr, Gi, -Gi)
    gmatB = np.stack([Gr, Gi, Gn], axis=0)

    # quad-interleaved image for the bilinear gather:
    # imgq[x*256+y] = [r(x,y), i(x,y), r(x+1,y), i(x+1,y),
    #                  r(x,y+1), i(x,y+1), r(x+1,y+1), i(x+1,y+1)]
    r = image_real.astype(np.float32)
    im = image_imag.astype(np.float32)
    rx = np.concatenate([r[1:], r[-1:]], axis=0)
    ix = np.concatenate([im[1:], im[-1:]], axis=0)
    ry = np.concatenate([r[:, 1:], r[:, -1:]], axis=1)
    iy = np.concatenate([im[:, 1:], im[:, -1:]], axis=1)
    rxy = np.concatenate([ry[1:], ry[-1:]], axis=0)
    ixy = np.concatenate([iy[1:], iy[-1:]], axis=0)
    imgq = np.stack([r, im, rx, ix, ry, iy, rxy, ixy], axis=-1)
    imgq = np.ascontiguousarray(imgq.reshape(Nx * Ny, 8))

    iotax = np.broadcast_to(
        np.arange(Nx, dtype=np.float32).reshape(2, 128)[:, :, None], (2, 128, Ny)
    )
    iotay = np.broadcast_to(np.arange(Ny, dtype=np.float32)[None, :], (128, Ny))
    return {
        "gmatA": gmatA,
        "gmatB": np.ascontiguousarray(gmatB),
        "imgq": imgq,
        "iotax": np.ascontiguousarray(iotax),
        "iotay": np.ascontiguousarray(iotay),
    }


def _shard_inputs(image_real, image_imag, mask, smaps_real, smaps_imag, flow):
    consts = _host_constants(image_real, image_imag)
    smapsT = np.ascontiguousarray(
        np.stack([smaps_real, smaps_imag], axis=0).transpose(3, 0, 1, 2)
    )  # [c, ri, x, y]
    in_maps = []
    for core in range(NCORES):
        ts = range(core * TPC, (core + 1) * TPC)
        fl = np.ascontiguousarray(
            np.stack([np.stack([flow[:, :, 0, t], flow[:, :, 1, t]]) for t in ts])
        )  # [tt, comp, x, y]
        mk = np.ascontiguousarray(
            np.stack(
                [np.stack([mask[:, :, c, t].T for t in ts]) for c in range(Nc)]
            )
        )  # [c, tt, ky(y), kx(x)]
        m = dict(consts)
        m["flow"] = fl
        m["maskt"] = mk
        m["smapst"] = smapsT
        in_maps.append(m)
    return in_maps


def _unshard(partials):
    # partial: [128, c, ri, m, kx]; ky = m*128 + p
    total = np.zeros_like(partials[0], dtype=np.float64)
    for p in partials:
        total += p
    total = total.astype(np.float32)
    # -> [ri, kx, ky, c]
    t = total.transpose(2, 4, 3, 0, 1)  # [ri, kx, m, p, c]
    t = t.reshape(2, Nx, Ny, Nc)
    return t


# -------------------------------------------------------------- kernel build

def _emit(nc, tc, debug=False):
    import concourse.mybir as mybir
    from concourse.bass import IndirectOffsetOnAxis

    f32 = mybir.dt.float32
    f32r = mybir.dt.float32r
    i32 = mybir.dt.int32
    Alu = mybir.AluOpType

    flow_d = nc.dram_tensor("flow", (TPC, 2, Nx, Ny), f32, kind="ExternalInput").ap()
    maskt_d = nc.dram_tensor("maskt", (Nc, TPC, Ny, Nx), f32, kind="ExternalInput").ap()
    smapst_d = nc.dram_tensor("smapst", (Nc, 2, Nx, Ny), f32, kind="ExternalInput").ap()
    gmatA_d = nc.dram_tensor("gmatA", (2, Nx, 2 * Ny), f32, kind="ExternalInput").ap()
    gmatB_d = nc.dram_tensor("gmatB", (3, Nx, Ny), f32, kind="ExternalInput").ap()
    imgq_d = nc.dram_tensor("imgq", (Nx * Ny, 8), f32, kind="ExternalInput").ap()
    iotax_d = nc.dram_tensor("iotax", (2, 128, Ny), f32, kind="ExternalInput").ap()
    iotay_d = nc.dram_tensor("iotay", (128, Ny), f32, kind="ExternalInput").ap()
    out_d = nc.dram_tensor(
        "out", (128, Nc, 2, 2, Ny), f32, kind="ExternalOutput"
    ).ap()
    if debug:
        dbg_qi = nc.dram_tensor("dbg_qi", (128, 2, Ny), mybir.dt.int32, kind="ExternalOutput").ap()
        dbg_quad = nc.dram_tensor("dbg_quad", (128, 2, Ny, 8), f32, kind="ExternalOutput").ap()
        dbg_W = nc.dram_tensor("dbg_W", (128, TPC, 2, 2, Ny), f32, kind="ExternalOutput").ap()
        dbg_X = nc.dram_tensor("dbg_X", (128, 2, 2, Ny), f32, kind="ExternalOutput").ap()
        dbg_s1 = nc.dram_tensor("dbg_s1", (128, 2, 2 * Ny), f32, kind="ExternalOutput").ap()
        dbg_kt = nc.dram_tensor("dbg_kt", (128, 2, 2 * Ny), f32, kind="ExternalOutput").ap()

    import contextlib

    ctx = contextlib.ExitStack()
    with ctx:
        consts = ctx.enter_context(tc.tile_pool(name="consts", bufs=1))
        accp = ctx.enter_context(tc.tile_pool(name="acc", bufs=1))
        warp = ctx.enter_context(tc.tile_pool(name="warp", bufs=2))
        wres = ctx.enter_context(tc.tile_pool(name="wres", bufs=1))
        unit = ctx.enter_context(tc.tile_pool(name="unit", bufs=2))
        mpool = ctx.enter_context(tc.tile_pool(name="mask", bufs=3))
        psA = ctx.enter_context(tc.tile_pool(name="psA", bufs=2, space="PSUM"))
        psB = ctx.enter_context(tc.tile_pool(name="psB", bufs=2, space="PSUM"))

        # ---- constants into SBUF
        gA = consts.tile([128, 2, 2, 2 * Ny], f32r, tag="gA")  # [p, var, ktile, 512]
        for v in range(2):
            nc.sync.dma_start(
                out=gA[:, v],
                in_=gmatA_d[v].rearrange("(k p) n -> p k n", p=128).bitcast(f32r),
            )
        gB = consts.tile([128, 3, 2, Ny], f32r, tag="gB")  # [p, plane, ktile, ky]
        for pl in range(3):
            nc.sync.dma_start(
                out=gB[:, pl],
                in_=gmatB_d[pl].rearrange("(k p) n -> p k n", p=128).bitcast(f32r),
            )
        iox = consts.tile([128, 2, Ny], f32, tag="iox")
        nc.sync.dma_start(out=iox, in_=iotax_d.rearrange("k p n -> p k n"))
        ioy = consts.tile([128, Ny], f32, tag="ioy")
        nc.sync.dma_start(out=ioy, in_=iotay_d)

        # ---- accumulator
        acc = accp.tile([128, Nc, 2, 2, Ny], f32, tag="acc")

        # ---- phase 1: warp each frame
        W = wres.tile([128, TPC, 2, 2, Ny], f32, tag="W")  # [p, tt, xt, ri, y]
        for tt in range(TPC):
            fx = warp.tile([128, 2, Ny], f32, tag="fx")
            fy = warp.tile([128, 2, Ny], f32, tag="fy")
            nc.sync.dma_start(
                out=fx, in_=flow_d[tt, 0].rearrange("(k p) n -> p k n", p=128)
            )
            nc.sync.dma_start(
                out=fy, in_=flow_d[tt, 1].rearrange("(k p) n -> p k n", p=128)
            )
            quads = []
            wxs = []
            wys = []
            for xt in range(2):
                xc = warp.tile([128, Ny], f32, tag="xc")
                yc = warp.tile([128, Ny], f32, tag="yc")
                nc.vector.tensor_tensor(out=xc, in0=fx[:, xt], in1=iox[:, xt], op=Alu.add)
                nc.vector.tensor_scalar(
                    out=xc, in0=xc, scalar1=0.0, scalar2=float(Nx - 1),
                    op0=Alu.max, op1=Alu.min,
                )
                nc.vector.tensor_tensor(out=yc, in0=fy[:, xt], in1=ioy, op=Alu.add)
                nc.vector.tensor_scalar(
                    out=yc, in0=yc, scalar1=0.0, scalar2=float(Ny - 1),
                    op0=Alu.max, op1=Alu.min,
                )
                # floor(x) = (x + (1.5*2^23 - 0.5)) - 1.5*2^23 under f32 RNE;
                # the tie direction is harmless: off-by-one floor gives w=1,
                # which selects the exact neighbor value in the lerp.
                MAGIC = 12582912.0
                x0 = warp.tile([128, Ny], f32, tag="x0")
                y0 = warp.tile([128, Ny], f32, tag="y0")
                nc.vector.tensor_single_scalar(out=x0, in_=xc, scalar=-0.5, op=Alu.add)
                nc.vector.tensor_single_scalar(out=x0, in_=x0, scalar=MAGIC, op=Alu.add)
                nc.vector.tensor_scalar(
                    out=x0, in0=x0, scalar1=MAGIC, scalar2=float(Nx - 2),
                    op0=Alu.subtract, op1=Alu.min,
                )
                nc.vector.tensor_single_scalar(out=y0, in_=yc, scalar=-0.5, op=Alu.add)
                nc.vector.tensor_single_scalar(out=y0, in_=y0, scalar=MAGIC, op=Alu.add)
                nc.vector.tensor_scalar(
                    out=y0, in0=y0, scalar1=MAGIC, scalar2=float(Ny - 2),
                    op0=Alu.subtract, op1=Alu.min,
                )
                # weights
                wx = warp.tile([128, Ny], f32, tag="wx")
                wy = warp.tile([128, Ny], f32, tag="wy")
                nc.vector.tensor_tensor(out=wx, in0=xc, in1=x0, op=Alu.subtract)
                nc.vector.tensor_tensor(out=wy, in0=yc, in1=y0, op=Alu.subtract)
                wxs.append(wx)
                wys.append(wy)
                # flat quad index
                qf = warp.tile([128, Ny], f32, tag="qf")
                nc.gpsimd.tensor_single_scalar(out=qf, in_=x0, scalar=float(Ny), op=Alu.mult)
                nc.gpsimd.tensor_tensor(out=qf, in0=qf, in1=y0, op=Alu.add)
                qi = warp.tile([128, Ny], i32, tag="qi")
                nc.gpsimd.tensor_copy(out=qi, in_=qf)
                quad = warp.tile([128, Ny, 8], f32, tag="quad")
                # HW honors only one dynamic offset per partition per DMA
                # (scalar_dynamic_offset DGE level): gather column-by-column.
                for j in range(Ny):
                    nc.gpsimd.indirect_dma_start(
                        out=quad[:, j], out_offset=None,
                        in_=imgq_d,
                        in_offset=IndirectOffsetOnAxis(ap=qi[:, j : j + 1], axis=0),
                    )
                if debug and tt == 0:
                    nc.sync.dma_start(out=dbg_qi[:, xt], in_=qi)
                    nc.sync.dma_start(out=dbg_quad[:, xt], in_=quad)
                quads.append(quad)
            for xt in range(2):
                quad, wx, wy = quads[xt], wxs[xt], wys[xt]
                # bilinear lerp -> W
                wu = warp.tile([128, Ny], f32, tag="wu")  # 1-wx
                wv = warp.tile([128, Ny], f32, tag="wv")  # 1-wy
                nc.vector.tensor_scalar(
                    out=wu, in0=wx, scalar1=-1.0, scalar2=1.0, op0=Alu.mult, op1=Alu.add
                )
                nc.vector.tensor_scalar(
                    out=wv, in0=wy, scalar1=-1.0, scalar2=1.0, op0=Alu.mult, op1=Alu.add
                )
                w00 = warp.tile([128, Ny], f32, tag="w00")
                w10 = warp.tile([128, Ny], f32, tag="w10")
                w01 = warp.tile([128, Ny], f32, tag="w01")
                w11 = warp.tile([128, Ny], f32, tag="w11")
                nc.vector.tensor_tensor(out=w00, in0=wu, in1=wv, op=Alu.mult)
                nc.vector.tensor_tensor(out=w10, in0=wx, in1=wv, op=Alu.mult)
                nc.vector.tensor_tensor(out=w01, in0=wu, in1=wy, op=Alu.mult)
                nc.vector.tensor_tensor(out=w11, in0=wx, in1=wy, op=Alu.mult)
                t0 = warp.tile([128, Ny], f32, tag="t0")
                for ri in range(2):
                    wdst = W[:, tt, xt, ri]
                    nc.vector.tensor_tensor(out=wdst, in0=w00, in1=quad[:, :, 0 + ri], op=Alu.mult)
                    nc.vector.tensor_tensor(out=t0, in0=w10, in1=quad[:, :, 2 + ri], op=Alu.mult)
                    nc.vector.tensor_tensor(out=wdst, in0=wdst, in1=t0, op=Alu.add)
                    nc.vector.tensor_tensor(out=t0, in0=w01, in1=quad[:, :, 4 + ri], op=Alu.mult)
                    nc.vector.tensor_tensor(out=wdst, in0=wdst, in1=t0, op=Alu.add)
                    nc.vector.tensor_tensor(out=t0, in0=w11, in1=quad[:, :, 6 + ri], op=Alu.mult)
                    nc.vector.tensor_tensor(out=wdst, in0=wdst, in1=t0, op=Alu.add)

        if debug:
            nc.sync.dma_start(out=dbg_W, in_=W)
        # ---- phase 2: per (frame, coil): coil mult, DFT, mask-MAC
        for tt in range(TPC):
            for c in range(Nc):
                smap = unit.tile([128, 2, 2, Ny], f32, tag="smap")  # [p, ri, xt, y]
                for ri in range(2):
                    nc.sync.dma_start(
                        out=smap[:, ri],
                        in_=smapst_d[c, ri].rearrange("(k p) n -> p k n", p=128),
                    )
                X = unit.tile([128, 2, 2, Ny], f32r, tag="X")  # [p, xt, ri, y]
                t1 = unit.tile([128, Ny], f32, tag="t1")
                t2 = unit.tile([128, Ny], f32, tag="t2")
                for xt in range(2):
                    wr = W[:, tt, xt, 0]
                    wi = W[:, tt, xt, 1]
                    sr = smap[:, 0, xt]
                    si = smap[:, 1, xt]
                    nc.vector.tensor_tensor(out=t1, in0=wr, in1=sr, op=Alu.mult)
                    nc.vector.tensor_tensor(out=t2, in0=wi, in1=si, op=Alu.mult)
                    nc.vector.tensor_tensor(out=X[:, xt, 0], in0=t1, in1=t2, op=Alu.subtract)
                    nc.vector.tensor_tensor(out=t1, in0=wr, in1=si, op=Alu.mult)
                    nc.vector.tensor_tensor(out=t2, in0=wi, in1=sr, op=Alu.mult)
                    nc.vector.tensor_tensor(out=X[:, xt, 1], in0=t1, in1=t2, op=Alu.add)

                # stage A: S1T[y, kx(r|i)] = sum_x X[x,y] * G[x,kx]
                pa = [psA.tile([128, 2 * Ny], f32, tag=f"psA{m}", name=f"psA{m}") for m in range(2)]
                for m in range(2):
                    ms = slice(m * 128, (m + 1) * 128)
                    for k in range(2):
                        nc.tensor.matmul(
                            pa[m][:],
                            X[:, k, 0, ms],
                            gA[:, 0, k],
                            start=(k == 0), stop=False,
                        )
                        nc.tensor.matmul(
                            pa[m][:],
                            X[:, k, 1, ms],
                            gA[:, 1, k],
                            start=False, stop=(k == 1),
                        )
                s1 = unit.tile([128, 2, 2 * Ny], f32r, tag="s1")  # [p, ytile, kxr|kxi]
                for m in range(2):
                    nc.scalar.copy(out=s1[:, m], in_=pa[m][:])
                if debug and c == 0 and tt == 0:
                    nc.sync.dma_start(out=dbg_X, in_=X.bitcast(f32))
                    nc.sync.dma_start(out=dbg_s1, in_=s1.bitcast(f32))

                # stage B: KT[ky, kx] = sum_y G[y,ky] * S1T[y,kx]
                pb = [psB.tile([128, 2 * Ny], f32, tag=f"psB{m}", name=f"psB{m}") for m in range(2)]
                for m2 in range(2):
                    ms = slice(m2 * 128, (m2 + 1) * 128)
                    # real half: Gr@S1Tr + (-Gi)@S1Ti  (planes 0, 2)
                    # imag half: Gi@S1Tr + Gr@S1Ti     (planes 1, 0)
                    for half, (pl_r, pl_i) in enumerate([(0, 2), (1, 0)]):
                        dst = pb[m2][:, half * Ny : (half + 1) * Ny]
                        for k2 in range(2):
                            nc.tensor.matmul(
                                dst,
                                gB[:, pl_r, k2, ms],
                                s1[:, k2, 0:Ny],
                                start=(k2 == 0), stop=False,
                            )
                            nc.tensor.matmul(
                                dst,
                                gB[:, pl_i, k2, ms],
                                s1[:, k2, Ny : 2 * Ny],
                                start=False, stop=(k2 == 1),
                            )

                if debug and c == 0 and tt == 0:
                    ktd = unit.tile([128, 2, 2 * Ny], f32, tag="ktd")
                    for m2 in range(2):
                        nc.scalar.copy(out=ktd[:, m2], in_=pb[m2][:])
                    nc.sync.dma_start(out=dbg_kt, in_=ktd)
                # mask MAC
                mk = mpool.tile([128, 2, Nx], f32, tag="mk")  # [p, kytile, kx]
                nc.sync.dma_start(
                    out=mk,
                    in_=maskt_d[c, tt].rearrange("(k p) n -> p k n", p=128),
                )
                mt = unit.tile([128, Nx], f32, tag="mt")
                for m2 in range(2):
                    for ri in range(2):
                        src = pb[m2][:, ri * Ny : (ri + 1) * Ny]
                        dst = acc[:, c, ri, m2]
                        if tt == 0:
                            nc.vector.tensor_tensor(out=dst, in0=mk[:, m2], in1=src, op=Alu.mult)
                        else:
                            nc.vector.tensor_tensor(out=mt, in0=mk[:, m2], in1=src, op=Alu.mult)
                            nc.vector.tensor_tensor(out=dst, in0=dst, in1=mt, op=Alu.add)

        # ---- out
        for half in range(2):
            nc.sync.dma_start(
                out=out_d[:, half * 8 : (half + 1) * 8],
                in_=acc[:, half * 8 : (half + 1) * 8],
            )


def _build(debug=False):
    key = ("nc", debug)
    if key in _cache:
        return _cache[key]
    import concourse.bacc as bacc
    import concourse.tile as tile

    nc = bacc.Bacc("TRN2", target_bir_lowering=False, debug=False)
    with tile.TileContext(nc) as tc:
        _emit(nc, tc, debug=debug)
    nc.compile()
    _cache[key] = nc
    return nc


def kernel(
    image_real=None, image_imag=None, mask=None,
    smaps_real=None, smaps_imag=None, flow=None,
):
    from concourse import bass_utils

    image_real = np.asarray(image_real, dtype=np.float32)
    image_imag = np.asarray(image_imag, dtype=np.float32)
    mask = np.asarray(mask, dtype=np.float32)
    smaps_real = np.asarray(smaps_real, dtype=np.float32)
    smaps_imag = np.asarray(smaps_imag, dtype=np.float32)
    flow = np.asarray(flow, dtype=np.float32)

    in_maps = _shard_inputs(image_real, image_imag, mask, smaps_real, smaps_imag, flow)
    nc = _build()
    res = bass_utils.run_bass_kernel_spmd(nc, in_maps, core_ids=list(range(NCORES)))
    partials = [r["out"] for r in res.results]
    return _unshard(partials)
